# revision 44
# baseline (speedup 1.0000x reference)
"""Trainium2 Bass kernel for the sparse-attention CompiledTransformerLayer.

Math (derived from the reference):
  c0 = rowsum(mask0); attended = (mask0 @ x[:,:,0:16]) * r/(1-r), r = 1/(1+c0)
  out ch16:32 = attended @ W_o0.T
  out ch32    = c1 * W_o1[0,0], c1 = rowsum(mask1)
  out ch48:64 = a + b; 64:80 = a*b; 80:96 = (a > b), a = x ch0:16, b = ch16:32
  all other channels pass through from x (merged on the host).

Sharding: 8 cores = 4 batches x 2 query-halves (1024 queries each).

Key tricks:
  - nibble packing: the host packs BOTH masks for two adjacent keys into one
    byte  v = m0[2j] + 2*m0[2j+1] + 4*(m1[2j]+m1[2j+1]).  fp8e4 decodes bytes
    0..15 exactly as k*2^-9, so 1MB/core (instead of 4MB) carries all mask
    information.  The host also pre-transposes the packed array to
    [key-lane, query] layout, so every device load is a plain contiguous DMA
    (no xbar transposes at all).
  - two DVE shift/and ops per piece recover the mask0 even/odd key bit-planes
    (bytes 0x00/0x01 = fp8 0 / 2^-9).
  - matmuls are flipped vs the usual attention layout: a 128x128 mask^T block
    is the *stationary* operand and an 18-wide fp16 value vector
    [u*512 | 512 (-> c0) | -512*(1+e) (-> -g)] is the *moving* operand, so
    each matmul streams only 18 columns.  PSUM lands as [query, channel].
  - u = x[:,:,0:16] @ W_o0.T is precomputed on the host in fp16 (x512 to
    cancel the fp8 2^-9).
  - c1 falls out linearly: raw packed bytes matmul'd against +512 accumulate
    T = g + 4*c1 into the same psum column that the -g moving column drains,
    leaving exactly 4*c1;  count = (T-g) * (W_o1/4).
  - attended scale: w = 1/max(c0,1) (exact: S == 0 when c0 == 0).
  - psum: start=True resets more than the addressed region, so the whole
    accumulator is zeroed once by a dummy all-zeros matmul and every real
    matmul accumulates with start=False.
"""
import sys
sys.path.insert(0, "/opt/trn_rl_repo")
import numpy as np

import concourse.bass as bass
import concourse.mybir as mybir
from concourse import tile
from concourse.bass_utils import run_bass_kernel_spmd
from concourse.vector_clock import ScopedClock, VectorClock
from concourse.tile import add_dep_helper

B, S, D = 4, 2048, 128
QH = S // 2              # queries per core
NIB = QH // 128          # query blocks per core (8)
NQ = 8                   # key-lane groups of 128 (jj = 128q + p)
DT = mybir.dt
AL = mybir.AluOpType

# walrus codegen rejects instructions with many sem waits; the Tile tail
# drain accumulates one wait per touched proc. Emit one single-wait drain
# per proc instead.
def _patched_dab(self, tick_clock, wait_clock):
    ticks = list(tick_clock.global_clock)
    for i, t in enumerate(ticks):
        if t <= 0:
            continue
        part = [t if j == i else 0 for j, t in enumerate(ticks)]
        d = self.nc.sync.drain()
        wait_clock.add_sem_waits(d.ins, ScopedClock({None: VectorClock(part)}))
    self.nc.sync.drain()
    self.nc.all_engine_barrier()
    popped = self.nc._tile_sem_poison_stack.pop()
    assert popped is self._sem_poison
    self.nc.clear_and_free_semaphores(list(self.sems.allocated().values()))
    self.nc.all_engine_barrier()
tile.TileContext._drain_and_barrier = _patched_dab

# (byte offset, nq, q0, rows, row0, ib0): tail pieces shrink so the
# post-last-DMA straggler is small
PIECES = [(0, 4, 0, 1024, 0, 0),
          (4096, 2, 4, 1024, 0, 0),
          (6144, 2, 6, 512, 0, 0),
          (7168, 2, 6, 384, 512, 4),
          (7936, 2, 6, 128, 896, 7)]


def _build_program():
    nc = bass.Bass()
    mpt_d = nc.declare_dram_parameter("mpt", [128, 8192], DT.uint8, isOutput=False)
    blob_d = nc.declare_dram_parameter("blob", [128, 1120], DT.uint8, isOutput=False)
    outa_d = nc.declare_dram_parameter("outa", [128, NIB, 48], DT.float16, isOutput=True)
    outb_d = nc.declare_dram_parameter("outb", [128, NIB, 17], DT.float16, isOutput=True)

    with tile.TileContext(nc) as tc, \
         tc.tile_pool(name="const", bufs=1) as cpool, \
         tc.tile_pool(name="masks", bufs=1) as mpool, \
         tc.tile_pool(name="work", bufs=1) as wpool, \
         tc.tile_pool(name="ps", bufs=1, space="PSUM") as ps:

        blob_t = cpool.tile([128, 1120], DT.uint8)
        nc.sync.dma_start(blob_t[:], blob_d[:])
        # DVE-launder the blob so every consumer dep collapses onto DVE sems
        blob2 = cpool.tile([128, 548], DT.uint16)
        nc.vector.tensor_copy(blob2[:], blob_t[:].bitcast(DT.uint16)[:, 0:548])
        w3 = blob2[:, 0:288].bitcast(DT.float16).rearrange(
            "p (q e d) -> p q e d", q=NQ, e=2, d=18)
        xq = blob2[:, 288:544].bitcast(DT.float32).rearrange(
            "p (i c) -> p i c", i=NIB, c=16)
        cwo1 = blob2[:, 544:546].bitcast(DT.float32)
        cone = blob2[:, 546:547].bitcast(DT.float16)

        # zero the accumulator with one dummy all-zeros matmul; accumulate
        # with start=False everywhere (start=True resets beyond its region).
        # One tile holds S (cols 0:16 att, 16 c0) and C (col 17, T - g).
        P_ps = ps.tile([128, NIB, 18], DT.float32, tag="P", name="P")
        zmv = cpool.tile([128, 160], DT.bfloat16)
        nc.vector.memset(zmv[:], 0.0)
        nc.tensor.matmul(P_ps[:].rearrange("p a b -> p (a b)"), zmv[:, 0:128],
                         zmv[:, 0:NIB * 18], start=True, stop=False,
                         skip_group_check=True)

        for pi, (off, nq, q0, rows, row0, ib0) in enumerate(PIECES):
            W = nq * rows
            last = (pi == len(PIECES) - 1)
            mt = mpool.tile([128, W], DT.uint8, tag=f"mt{pi}", name=f"mt{pi}")
            nc.sync.dma_start(mt[:], mpt_d[:, off:off + W])
            # bit-plane extracts: even keys = bit0, odd keys = bit1 per byte
            mt16 = mt[:].bitcast(DT.uint16)
            ev = mpool.tile([128, W // 2], DT.uint16, tag=f"ev{pi}", name=f"ev{pi}")
            nc.vector.tensor_scalar(ev[:], mt16, 0x0101, 0, AL.bitwise_and,
                                    AL.bitwise_or)
            od = mpool.tile([128, W // 2], DT.uint16, tag=f"od{pi}", name=f"od{pi}")
            odx = nc.vector.tensor_scalar(od[:], mt16, 1, 0x0101,
                                          AL.logical_shift_right, AL.bitwise_and)

            mr = mt[:].bitcast(DT.float8e4).rearrange("p (q i) -> p q i", q=nq)
            evr = ev[:].bitcast(DT.float8e4).rearrange("p (q i) -> p q i", q=nq)
            odr = od[:].bitcast(DT.float8e4).rearrange("p (q i) -> p q i", q=nq)

            nib = rows // 128
            for qq in range(nq):
                # 18-wide fp16 moving: [u (16) | ones -> c0 | -(1+e) -> -g]
                for e, pl in ((0, evr), (1, odr)):
                    stops = (last and qq == nq - 1 and e == 1)
                    mv = w3[:, q0 + qq, e, :]
                    for k in range(nib):
                        nc.tensor.matmul(
                            P_ps[:, ib0 + k, 0:18],
                            pl[:, qq, 128 * k:128 * (k + 1)], mv,
                            start=False,
                            stop=(stops and k == nib - 1),
                            skip_group_check=True)
                # raw packed bytes vs +512 ones: C += g + 4*c1
                for k in range(nib):
                    cmm = nc.tensor.matmul(
                        P_ps[:, ib0 + k, 17:18],
                        mr[:, qq, 128 * k:128 * (k + 1)], cone[:, 0:1],
                        start=False,
                        stop=(last and qq == nq - 1 and k == nib - 1),
                        skip_group_check=True)
                    # route deps through the piece's DVE extract so the wait
                    # set collapses to a single DVE sem (covers mt + cone)
                    add_dep_helper(cmm.ins, odx.ins, reason="piece ready")

        # ---- tail: scale + MLP, [128 queries, NIB, ch] layout ------------
        # attended = S * w with w = 1/max(c0, 1): exact for c0 >= 1, and for
        # c0 == 0 S is exactly 0 so any finite w gives the reference 0.
        # otA: [atts 16 | a+b 16 | a*b 16]; otB: [count 1 | a>b 16] (fp16:
        # count <= 2047 and the 0/1 comparison bits are exact in fp16).
        # storeA issues after mult so its DGE prep overlaps islt/count.
        mcol = wpool.tile([128, NIB], DT.float32, tag="mcol")
        nc.vector.tensor_scalar_max(mcol[:], P_ps[:, :, 16], 1.0)
        wcol = wpool.tile([128, NIB], DT.float32, tag="wcol")
        nc.vector.reciprocal(wcol[:], mcol[:])
        otA = wpool.tile([128, NIB, 48], DT.float16, tag="otA")
        otB = wpool.tile([128, NIB, 17], DT.float16, tag="otB")
        atts = wpool.tile([128, NIB, 16], DT.float32, tag="atts")
        wb = wcol[:].unsqueeze(2).broadcast_to([128, NIB, 16])
        nc.vector.tensor_tensor(atts[:], P_ps[:, :, 0:16], wb, AL.mult)
        nc.vector.tensor_copy(otA[:, :, 0:16], atts[:])
        # MLP: a = x ch0:16, b = attended (kept f32 for exact comparisons)
        nc.vector.tensor_tensor(otA[:, :, 16:32], atts[:], xq, AL.add)
        nc.vector.tensor_tensor(otA[:, :, 32:48], atts[:], xq, AL.mult)
        nc.sync.dma_start(outa_d[:], otA[:])
        nc.vector.tensor_tensor(otB[:, :, 1:17], atts[:], xq, AL.is_lt)
        # count: c1*W_o1 = (T - g) * (W_o1/4), C psum already holds T - g
        nc.vector.scalar_tensor_tensor(otB[:, :, 0], P_ps[:, :, 17], cwo1,
                                       wcol[:], AL.mult, AL.bypass)
        # 6 loads + 2 stores = 8 DMAs on SP, one per HWDGE queue: each store
        # carries only its DVE data-dep sem (walrus allows one per DMA)
        nc.sync.dma_start(outb_d[:], otB[:])

    return nc


_cached = {}


def _prepare_in_maps(x, mask0, mask1, W_o0, W_o1):
    x = np.asarray(x, dtype=np.float32)
    m0 = np.asarray(mask0).view(np.uint8)
    m1 = np.asarray(mask1).view(np.uint8)
    W_o0 = np.asarray(W_o0, dtype=np.float32)
    W_o1 = np.asarray(W_o1, dtype=np.float32)

    # nibble pack: byte jj = m0[2jj] + 2*m0[2jj+1] + 4*(m1[2jj]+m1[2jj+1])
    packed = (m0[..., 0::2] + (m0[..., 1::2] << 1)
              + ((m1[..., 0::2] + m1[..., 1::2]) << 2))        # (B, S, S//2) u8

    # u = values through the head-0 output projection, fp16 x512
    u = x[:, :, 0:16] @ W_o0.T                                 # (B, S, 16) f32
    u16 = (512.0 * u).astype(np.float16)

    # key index per (partition, lane-group, parity): j = 256q + 2p + e
    p_i = np.arange(128)[:, None, None]
    q_i = np.arange(NQ)[None, :, None]
    e_i = np.arange(2)[None, None, :]
    J = 256 * q_i + 2 * p_i + e_i                              # [128, 8, 2]

    cwo1 = np.full((128, 1), float(W_o1[0, 0]) / 4.0, dtype=np.float32)
    cone = np.full((128, 1), 512.0, dtype=np.float16)

    blobs = []
    for b in range(B):
        w3 = np.zeros((128, NQ, 2, 18), dtype=np.float16)
        w3[..., 0:16] = u16[b][J]
        w3[..., 16] = 512.0                                    # ones -> c0
        w3[..., 17] = -512.0 * (1.0 + e_i[0])                  # -g accumulation
        blobs.append(w3)

    in_maps = []
    for c in range(8):
        b, h = divmod(c, 2)
        sl = slice(QH * h, QH * (h + 1))
        xq = np.ascontiguousarray(
            x[b, sl, 0:16].reshape(NIB, 128, 16).transpose(1, 0, 2))
        blob = np.zeros((128, 1120), np.uint8)
        blob[:, 0:576] = blobs[b].reshape(128, 288).view(np.uint8)
        blob[:, 576:1088] = xq.reshape(128, 128).view(np.uint8)
        blob[:, 1088:1092] = cwo1.view(np.uint8)
        blob[:, 1092:1094] = cone.view(np.uint8)
        # host-transposed packed masks: A[p, q, i] = packed[i, 128q + p]
        A = np.ascontiguousarray(
            packed[b, sl, :].T.reshape(NQ, 128, QH).transpose(1, 0, 2))
        mpt = np.concatenate(
            [A[:, q0:q0 + nq, row0:row0 + rows].reshape(128, -1)
             for (off, nq, q0, rows, row0, ib0) in PIECES], axis=1)
        in_maps.append({"mpt": np.ascontiguousarray(mpt), "blob": blob})
    return in_maps


def kernel(x, mask0, mask1, W_o0, W_o1):
    if "nc" not in _cached:
        _cached["nc"] = _build_program()
    nc = _cached["nc"]
    in_maps = _prepare_in_maps(x, mask0, mask1, W_o0, W_o1)
    res = run_bass_kernel_spmd(nc, in_maps, list(range(8)))
    _cached["last_results"] = res
    out = np.array(np.asarray(x, dtype=np.float32), copy=True)
    for c in range(8):
        b, h = divmod(c, 2)
        sl = slice(QH * h, QH * (h + 1))
        ra = res.results[c]["outa"].astype(np.float32).transpose(1, 0, 2)
        ra = ra.reshape(QH, 48)
        rb = res.results[c]["outb"].astype(np.float32).transpose(1, 0, 2)
        rb = rb.reshape(QH, 17)
        out[b, sl, 16:32] = ra[:, 0:16]
        out[b, sl, 48:64] = ra[:, 16:32]
        out[b, sl, 64:80] = ra[:, 32:48]
        out[b, sl, 32] = rb[:, 0]
        out[b, sl, 80:96] = rb[:, 1:17]
    return out


# revision 45
# speedup vs baseline: 1.0546x; 1.0546x over previous
"""Trainium2 Bass kernel for the sparse-attention CompiledTransformerLayer.

Math (derived from the reference):
  c0 = rowsum(mask0); attended = (mask0 @ x[:,:,0:16]) * r/(1-r), r = 1/(1+c0)
  out ch16:32 = attended @ W_o0.T
  out ch32    = c1 * W_o1[0,0], c1 = rowsum(mask1)
  out ch48:64 = a + b; 64:80 = a*b; 80:96 = (a > b), a = x ch0:16, b = ch16:32
  all other channels pass through from x (merged on the host).

Sharding: 8 cores = 4 batches x 2 query-halves (1024 queries each).

Key tricks:
  - nibble packing: the host packs BOTH masks for two adjacent keys into one
    byte  v = m0[2j] + 2*m0[2j+1] + 4*(m1[2j]+m1[2j+1]).  fp8e4 decodes bytes
    0..15 exactly as k*2^-9, so 1MB/core (instead of 4MB) carries all mask
    information.  The host also pre-transposes the packed array to
    [key-lane, query] layout, so every device load is a plain contiguous DMA
    (no xbar transposes at all).
  - two DVE shift/and ops per piece recover the mask0 even/odd key bit-planes
    (bytes 0x00/0x01 = fp8 0 / 2^-9).
  - matmuls are flipped vs the usual attention layout: a 128x128 mask^T block
    is the *stationary* operand and an 18-wide fp16 value vector
    [u*512 | 512 (-> c0) | -512*(1+e) (-> -g)] is the *moving* operand, so
    each matmul streams only 18 columns.  PSUM lands as [query, channel].
  - u = x[:,:,0:16] @ W_o0.T is precomputed on the host in fp16 (x512 to
    cancel the fp8 2^-9).
  - c1 falls out linearly: raw packed bytes matmul'd against +512 accumulate
    T = g + 4*c1 into the same psum column that the -g moving column drains,
    leaving exactly 4*c1;  count = (T-g) * (W_o1/4).
  - attended scale: w = 1/max(c0,1) (exact: S == 0 when c0 == 0).
  - psum: start=True resets more than the addressed region, so the whole
    accumulator is zeroed once by a dummy all-zeros matmul and every real
    matmul accumulates with start=False.
"""
import sys
sys.path.insert(0, "/opt/trn_rl_repo")
import numpy as np

import concourse.bass as bass
import concourse.mybir as mybir
from concourse import tile
from concourse.bass_utils import run_bass_kernel_spmd
from concourse.vector_clock import ScopedClock, VectorClock
from concourse.tile import add_dep_helper

B, S, D = 4, 2048, 128
QH = S // 2              # queries per core
NIB = QH // 128          # query blocks per core (8)
NQ = 8                   # key-lane groups of 128 (jj = 128q + p)
DT = mybir.dt
AL = mybir.AluOpType

# walrus codegen rejects instructions with many sem waits; the Tile tail
# drain accumulates one wait per touched proc. Emit one single-wait drain
# per proc instead.
def _patched_dab(self, tick_clock, wait_clock):
    ticks = list(tick_clock.global_clock)
    for i, t in enumerate(ticks):
        if t <= 0:
            continue
        part = [t if j == i else 0 for j, t in enumerate(ticks)]
        d = self.nc.sync.drain()
        wait_clock.add_sem_waits(d.ins, ScopedClock({None: VectorClock(part)}))
    self.nc.sync.drain()
    self.nc.all_engine_barrier()
    popped = self.nc._tile_sem_poison_stack.pop()
    assert popped is self._sem_poison
    self.nc.clear_and_free_semaphores(list(self.sems.allocated().values()))
    self.nc.all_engine_barrier()
tile.TileContext._drain_and_barrier = _patched_dab

# (byte offset, nq, q0, rows, row0, ib0): tail pieces shrink so the
# post-last-DMA straggler is small
PIECES = [(0, 2, 0, 1024, 0, 0),
          (2048, 2, 2, 1024, 0, 0),
          (4096, 2, 4, 1024, 0, 0),
          (6144, 2, 6, 512, 0, 0),
          (7168, 2, 6, 512, 512, 4)]


def _build_program():
    nc = bass.Bass()
    mpt_d = nc.declare_dram_parameter("mpt", [128, 8192], DT.uint8, isOutput=False)
    blob_d = nc.declare_dram_parameter("blob", [128, 1120], DT.uint8, isOutput=False)
    outa_d = nc.declare_dram_parameter("outa", [128, NIB, 48], DT.float16, isOutput=True)
    outb_d = nc.declare_dram_parameter("outb", [128, NIB, 17], DT.float16, isOutput=True)

    with tile.TileContext(nc) as tc, \
         tc.tile_pool(name="const", bufs=1) as cpool, \
         tc.tile_pool(name="masks", bufs=1) as mpool, \
         tc.tile_pool(name="work", bufs=1) as wpool, \
         tc.tile_pool(name="ps", bufs=1, space="PSUM") as ps:

        blob_t = cpool.tile([128, 1120], DT.uint8)
        nc.sync.dma_start(blob_t[:], blob_d[:])
        # DVE-launder the blob so every consumer dep collapses onto DVE sems
        blob2 = cpool.tile([128, 548], DT.uint16)
        nc.vector.tensor_copy(blob2[:], blob_t[:].bitcast(DT.uint16)[:, 0:548])
        w3 = blob2[:, 0:288].bitcast(DT.float16).rearrange(
            "p (q e d) -> p q e d", q=NQ, e=2, d=18)
        xq = blob2[:, 288:544].bitcast(DT.float32).rearrange(
            "p (i c) -> p i c", i=NIB, c=16)
        cwo1 = blob2[:, 544:546].bitcast(DT.float32)
        cone = blob2[:, 546:547].bitcast(DT.float16)

        # zero the accumulator with one dummy all-zeros matmul; accumulate
        # with start=False everywhere (start=True resets beyond its region).
        # One tile holds S (cols 0:16 att, 16 c0) and C (col 17, T - g).
        P_ps = ps.tile([128, NIB, 18], DT.float32, tag="P", name="P")
        zmv = cpool.tile([128, 160], DT.bfloat16)
        nc.vector.memset(zmv[:], 0.0)
        nc.tensor.matmul(P_ps[:].rearrange("p a b -> p (a b)"), zmv[:, 0:128],
                         zmv[:, 0:NIB * 18], start=True, stop=False,
                         skip_group_check=True)

        for pi, (off, nq, q0, rows, row0, ib0) in enumerate(PIECES):
            W = nq * rows
            last = (pi == len(PIECES) - 1)
            mt = mpool.tile([128, W], DT.uint8, tag=f"mt{pi}", name=f"mt{pi}")
            nc.sync.dma_start(mt[:], mpt_d[:, off:off + W])
            # bit-plane extracts: even keys = bit0, odd keys = bit1 per byte
            mt16 = mt[:].bitcast(DT.uint16)
            ev = mpool.tile([128, W // 2], DT.uint16, tag=f"ev{pi}", name=f"ev{pi}")
            nc.vector.tensor_scalar(ev[:], mt16, 0x0101, 0, AL.bitwise_and,
                                    AL.bitwise_or)
            od = mpool.tile([128, W // 2], DT.uint16, tag=f"od{pi}", name=f"od{pi}")
            odx = nc.vector.tensor_scalar(od[:], mt16, 1, 0x0101,
                                          AL.logical_shift_right, AL.bitwise_and)

            mr = mt[:].bitcast(DT.float8e4).rearrange("p (q i) -> p q i", q=nq)
            evr = ev[:].bitcast(DT.float8e4).rearrange("p (q i) -> p q i", q=nq)
            odr = od[:].bitcast(DT.float8e4).rearrange("p (q i) -> p q i", q=nq)

            nib = rows // 128
            for qq in range(nq):
                # 18-wide fp16 moving: [u (16) | ones -> c0 | -(1+e) -> -g]
                for e, pl in ((0, evr), (1, odr)):
                    stops = (last and qq == nq - 1 and e == 1)
                    mv = w3[:, q0 + qq, e, :]
                    for k in range(nib):
                        nc.tensor.matmul(
                            P_ps[:, ib0 + k, 0:18],
                            pl[:, qq, 128 * k:128 * (k + 1)], mv,
                            start=False,
                            stop=(stops and k == nib - 1),
                            skip_group_check=True)
                # raw packed bytes vs +512 ones: C += g + 4*c1
                for k in range(nib):
                    cmm = nc.tensor.matmul(
                        P_ps[:, ib0 + k, 17:18],
                        mr[:, qq, 128 * k:128 * (k + 1)], cone[:, 0:1],
                        start=False,
                        stop=(last and qq == nq - 1 and k == nib - 1),
                        skip_group_check=True)
                    # route deps through the piece's DVE extract so the wait
                    # set collapses to a single DVE sem (covers mt + cone)
                    add_dep_helper(cmm.ins, odx.ins, reason="piece ready")

        # ---- tail: scale + MLP, [128 queries, NIB, ch] layout ------------
        # attended = S * w with w = 1/max(c0, 1): exact for c0 >= 1, and for
        # c0 == 0 S is exactly 0 so any finite w gives the reference 0.
        # otA: [atts 16 | a+b 16 | a*b 16]; otB: [count 1 | a>b 16] (fp16:
        # count <= 2047 and the 0/1 comparison bits are exact in fp16).
        # storeA issues after mult so its DGE prep overlaps islt/count.
        mcol = wpool.tile([128, NIB], DT.float32, tag="mcol")
        nc.vector.tensor_scalar_max(mcol[:], P_ps[:, :, 16], 1.0)
        wcol = wpool.tile([128, NIB], DT.float32, tag="wcol")
        nc.vector.reciprocal(wcol[:], mcol[:])
        otA = wpool.tile([128, NIB, 48], DT.float16, tag="otA")
        otB = wpool.tile([128, NIB, 17], DT.float16, tag="otB")
        atts = wpool.tile([128, NIB, 16], DT.float32, tag="atts")
        wb = wcol[:].unsqueeze(2).broadcast_to([128, NIB, 16])
        nc.vector.tensor_tensor(atts[:], P_ps[:, :, 0:16], wb, AL.mult)
        nc.vector.tensor_copy(otA[:, :, 0:16], atts[:])
        # count: c1*W_o1 = (T - g) * (W_o1/4), C psum already holds T - g
        nc.vector.scalar_tensor_tensor(otB[:, :, 0], P_ps[:, :, 17], cwo1,
                                       wcol[:], AL.mult, AL.bypass)
        nc.vector.tensor_tensor(otB[:, :, 1:17], atts[:], xq, AL.is_lt)
        sb = nc.sync.dma_start(outb_d[:], otB[:])
        # MLP: a = x ch0:16, b = attended (kept f32 for exact comparisons)
        nc.vector.tensor_tensor(otA[:, :, 16:32], atts[:], xq, AL.add)
        nc.vector.tensor_tensor(otA[:, :, 32:48], atts[:], xq, AL.mult)
        # 6 loads + 2 stores = 8 DMAs on SP, one per HWDGE queue: each store
        # carries only its DVE data-dep sem (walrus allows one per DMA)
        nc.sync.dma_start(outa_d[:], otA[:])

    return nc


_cached = {}


def _prepare_in_maps(x, mask0, mask1, W_o0, W_o1):
    x = np.asarray(x, dtype=np.float32)
    m0 = np.asarray(mask0).view(np.uint8)
    m1 = np.asarray(mask1).view(np.uint8)
    W_o0 = np.asarray(W_o0, dtype=np.float32)
    W_o1 = np.asarray(W_o1, dtype=np.float32)

    # nibble pack: byte jj = m0[2jj] + 2*m0[2jj+1] + 4*(m1[2jj]+m1[2jj+1])
    packed = (m0[..., 0::2] + (m0[..., 1::2] << 1)
              + ((m1[..., 0::2] + m1[..., 1::2]) << 2))        # (B, S, S//2) u8

    # u = values through the head-0 output projection, fp16 x512
    u = x[:, :, 0:16] @ W_o0.T                                 # (B, S, 16) f32
    u16 = (512.0 * u).astype(np.float16)

    # key index per (partition, lane-group, parity): j = 256q + 2p + e
    p_i = np.arange(128)[:, None, None]
    q_i = np.arange(NQ)[None, :, None]
    e_i = np.arange(2)[None, None, :]
    J = 256 * q_i + 2 * p_i + e_i                              # [128, 8, 2]

    cwo1 = np.full((128, 1), float(W_o1[0, 0]) / 4.0, dtype=np.float32)
    cone = np.full((128, 1), 512.0, dtype=np.float16)

    blobs = []
    for b in range(B):
        w3 = np.zeros((128, NQ, 2, 18), dtype=np.float16)
        w3[..., 0:16] = u16[b][J]
        w3[..., 16] = 512.0                                    # ones -> c0
        w3[..., 17] = -512.0 * (1.0 + e_i[0])                  # -g accumulation
        blobs.append(w3)

    in_maps = []
    for c in range(8):
        b, h = divmod(c, 2)
        sl = slice(QH * h, QH * (h + 1))
        xq = np.ascontiguousarray(
            x[b, sl, 0:16].reshape(NIB, 128, 16).transpose(1, 0, 2))
        blob = np.zeros((128, 1120), np.uint8)
        blob[:, 0:576] = blobs[b].reshape(128, 288).view(np.uint8)
        blob[:, 576:1088] = xq.reshape(128, 128).view(np.uint8)
        blob[:, 1088:1092] = cwo1.view(np.uint8)
        blob[:, 1092:1094] = cone.view(np.uint8)
        # host-transposed packed masks: A[p, q, i] = packed[i, 128q + p]
        A = np.ascontiguousarray(
            packed[b, sl, :].T.reshape(NQ, 128, QH).transpose(1, 0, 2))
        mpt = np.concatenate(
            [A[:, q0:q0 + nq, row0:row0 + rows].reshape(128, -1)
             for (off, nq, q0, rows, row0, ib0) in PIECES], axis=1)
        in_maps.append({"mpt": np.ascontiguousarray(mpt), "blob": blob})
    return in_maps


def kernel(x, mask0, mask1, W_o0, W_o1):
    if "nc" not in _cached:
        _cached["nc"] = _build_program()
    nc = _cached["nc"]
    in_maps = _prepare_in_maps(x, mask0, mask1, W_o0, W_o1)
    res = run_bass_kernel_spmd(nc, in_maps, list(range(8)))
    _cached["last_results"] = res
    out = np.array(np.asarray(x, dtype=np.float32), copy=True)
    for c in range(8):
        b, h = divmod(c, 2)
        sl = slice(QH * h, QH * (h + 1))
        ra = res.results[c]["outa"].astype(np.float32).transpose(1, 0, 2)
        ra = ra.reshape(QH, 48)
        rb = res.results[c]["outb"].astype(np.float32).transpose(1, 0, 2)
        rb = rb.reshape(QH, 17)
        out[b, sl, 16:32] = ra[:, 0:16]
        out[b, sl, 48:64] = ra[:, 16:32]
        out[b, sl, 64:80] = ra[:, 32:48]
        out[b, sl, 32] = rb[:, 0]
        out[b, sl, 80:96] = rb[:, 1:17]
    return out
